# revision 5
# baseline (speedup 1.0000x reference)
"""GCN policy network forward on 8 Trainium2 NeuronCores (Bass/Tile).

Strategy (graph/data parallel, per sharding hint):
- 128 graphs -> 16 graphs per core; node slice per core at graph boundaries.
- Layer 1: edges partitioned by dst. Per-edge rows of x are fetched with
  dma_gather (fp32, 512B rows); per 256-node dst block, PE accumulates
  tmpT[in_feat, node] += Xg_chunk^T @ sel_chunk where sel is built on DVE
  from per-edge (dst_local, norm) scalars.  Then h1 = relu(tmpT^T @ W1 + b1)
  stays resident in SBUF.
- Layer 2: edges partitioned by src (same node slice), so messages read the
  core's own h1.  Pooling is linear, so each core scatters directly into a
  per-graph pooled accumulator: Tt[feat, graph] += h1_chunk^T @ selL2_chunk
  with selL2 (multi-hot, norm/cnt weighted) precomputed on host.  One 64KB
  AllReduce combines the partial pooled sums; the head matmul runs on every
  core identically.

The device run executes in a subprocess (crash isolation + retry).
"""

import json
import os
import subprocess
import sys
import tempfile

import numpy as np

N_NODES = 50000
N_EDGES = 800000
N_GRAPHS = 128
D = 128
N_ACT = 64
N_CORES = 8
GPC = N_GRAPHS // N_CORES  # graphs per core

BLK = 256  # dst nodes per L1 block (psum free dim)
WIN = 2048  # rows per dma_gather call
LO_LIM = 32768  # int16 index limit


# ---------------------------------------------------------------- host prep


def _prep(x, edge_index, batch, W1, b1, W2, b2, W_head, b_head):
    src = np.concatenate([edge_index[0], np.arange(N_NODES, dtype=np.int64)])
    dst = np.concatenate([edge_index[1], np.arange(N_NODES, dtype=np.int64)])
    deg = np.bincount(dst, minlength=N_NODES).astype(np.float32)
    dinv = 1.0 / np.sqrt(deg)  # every node has a self-loop -> deg >= 1
    norm = (dinv[src] * dinv[dst]).astype(np.float32)

    batch = np.asarray(batch, np.int64)
    gs = np.searchsorted(batch, np.arange(N_GRAPHS + 1))
    cnt = np.bincount(batch, minlength=N_GRAPHS).astype(np.float32)
    cntc = np.maximum(cnt, 1.0)

    core_n0 = [int(gs[GPC * c]) for c in range(N_CORES)]
    core_n1 = [int(gs[GPC * (c + 1)]) for c in range(N_CORES)]
    n_blk = max(-(-(core_n1[c] - core_n0[c]) // BLK) for c in range(N_CORES))

    # ---- L1: per-core edge lists grouped by (dst block, lo/hi, src) ----
    per_core = []
    a_need = b_need = 0
    for c in range(N_CORES):
        n0, n1 = core_n0[c], core_n1[c]
        em = (dst >= n0) & (dst < n1)
        s_, d_, w_ = src[em], dst[em] - n0, norm[em]
        blk = d_ // BLK
        ishi = (s_ >= LO_LIM).astype(np.int64)
        order = np.lexsort((s_, ishi, blk))
        s_, d_, w_, blk, ishi = (a[order] for a in (s_, d_, w_, blk, ishi))
        per_core.append((s_, d_, w_, blk, ishi, n0, n1))
        for b in range(n_blk):
            nlo = int(np.count_nonzero((blk == b) & (ishi == 0)))
            nhi = int(np.count_nonzero((blk == b) & (ishi == 1)))
            a_need = max(a_need, -(-nlo // 128))
            b_need = max(b_need, -(-nhi // 128))
    A, B = a_need, b_need

    def pad_to(n, m):
        return -(-n // m) * m

    n_lo_slots = n_blk * A
    n_hi_slots = n_blk * B
    lo_rows = pad_to(n_lo_slots * 128, WIN)
    hi_rows = pad_to(n_hi_slots * 128, WIN)
    n_win_lo = lo_rows // WIN
    n_win_hi = hi_rows // WIN

    def pack_idx(idx):
        # [n] -> [128, n//16] int16, wrapped in 16 partitions, replicated x8
        a = np.asarray(idx, np.int16).reshape(-1, 16).T
        return np.tile(a, (8, 1))

    in_maps = []
    for c in range(N_CORES):
        s_, d_, w_, blk, ishi, n0, n1 = per_core[c]
        loidx = np.zeros(lo_rows, np.int64)
        low = np.zeros(lo_rows, np.float32)
        lod = np.zeros(lo_rows, np.float32)
        hiidx = np.zeros(hi_rows, np.int64)
        hiw = np.zeros(hi_rows, np.float32)
        hid = np.zeros(hi_rows, np.float32)
        for b in range(n_blk):
            mlo = (blk == b) & (ishi == 0)
            mhi = (blk == b) & (ishi == 1)
            nlo, nhi = int(np.count_nonzero(mlo)), int(np.count_nonzero(mhi))
            o = b * A * 128
            loidx[o : o + nlo] = s_[mlo]
            low[o : o + nlo] = w_[mlo]
            lod[o : o + nlo] = d_[mlo] - b * BLK
            o = b * B * 128
            hiidx[o : o + nhi] = s_[mhi] - LO_LIM
            hiw[o : o + nhi] = w_[mhi]
            hid[o : o + nhi] = d_[mhi] - b * BLK

        # stream position q -> (chunk q//128, partition q%128)
        def to_pc(a, n_slots):
            return np.ascontiguousarray(
                a[: n_slots * 128].reshape(n_slots, 128).T
            )

        im = {
            "x": None,  # filled below (shared)
            "idx_lo": np.stack(
                [pack_idx(loidx[w * WIN : (w + 1) * WIN]) for w in range(n_win_lo)]
            ),
            "idx_hi": np.stack(
                [pack_idx(hiidx[w * WIN : (w + 1) * WIN]) for w in range(n_win_hi)]
            ),
            "w_lo": to_pc(low, n_lo_slots),
            "d_lo": to_pc(lod, n_lo_slots),
            "w_hi": to_pc(hiw, n_hi_slots),
            "d_hi": to_pc(hid, n_hi_slots),
        }

        # ---- L2: multi-hot sel over the core's own nodes ----
        em2 = (src >= n0) & (src < n1)
        ln = (src[em2] - n0).astype(np.int64)
        gg = batch[dst[em2]]
        ww = (norm[em2] / cntc[gg]).astype(np.float32)
        sel2 = np.zeros((128, 2 * n_blk, 128), np.float32)
        np.add.at(sel2, (ln % 128, ln // 128, gg), ww)
        im["sel2"] = sel2
        in_maps.append(im)

    x = np.ascontiguousarray(np.asarray(x, np.float32))
    iota = np.tile(np.arange(BLK, dtype=np.float32)[None, :], (128, 1))
    b1b = np.tile(np.asarray(b1, np.float32)[None, :], (128, 1))
    b2c = np.asarray(b2, np.float32)[:, None].copy()
    bhb = np.tile(np.asarray(b_head, np.float32)[None, :], (128, 1))
    ident = np.eye(128, dtype=np.float32)
    consts = {
        "x": x,
        "W1": np.ascontiguousarray(np.asarray(W1, np.float32)),
        "W2": np.ascontiguousarray(np.asarray(W2, np.float32)),
        "Wh": np.ascontiguousarray(np.asarray(W_head, np.float32)),
        "iota": iota,
        "b1b": b1b,
        "b2c": b2c,
        "bhb": bhb,
        "ident": ident,
    }
    for im in in_maps:
        im.update(consts)

    meta = {
        "n_blk": n_blk,
        "A": A,
        "B": B,
        "n_win_lo": n_win_lo,
        "n_win_hi": n_win_hi,
    }
    return in_maps, meta


# ------------------------------------------------------------- bass builder


def _build(meta, split=True):
    import concourse.bass as bass
    import concourse.mybir as mybir
    import concourse.tile as tile
    from concourse import bacc

    n_blk, A, B = meta["n_blk"], meta["A"], meta["B"]
    n_win_lo, n_win_hi = meta["n_win_lo"], meta["n_win_hi"]
    f32 = mybir.dt.float32

    nc = bacc.Bacc(None, num_devices=N_CORES, num_swdge_queues=1)

    x = nc.dram_tensor("x", [N_NODES, D], f32, kind="ExternalInput")
    idx_lo = nc.dram_tensor("idx_lo", [n_win_lo, 128, WIN // 16], mybir.dt.int16, kind="ExternalInput")
    idx_hi = nc.dram_tensor("idx_hi", [n_win_hi, 128, WIN // 16], mybir.dt.int16, kind="ExternalInput")
    w_lo = nc.dram_tensor("w_lo", [128, n_blk * A], f32, kind="ExternalInput")
    d_lo = nc.dram_tensor("d_lo", [128, n_blk * A], f32, kind="ExternalInput")
    w_hi = nc.dram_tensor("w_hi", [128, n_blk * B], f32, kind="ExternalInput")
    d_hi = nc.dram_tensor("d_hi", [128, n_blk * B], f32, kind="ExternalInput")
    sel2 = nc.dram_tensor("sel2", [128, 2 * n_blk, 128], f32, kind="ExternalInput")
    W1 = nc.dram_tensor("W1", [D, D], f32, kind="ExternalInput")
    W2 = nc.dram_tensor("W2", [D, D], f32, kind="ExternalInput")
    Wh = nc.dram_tensor("Wh", [D, N_ACT], f32, kind="ExternalInput")
    iota = nc.dram_tensor("iota", [128, BLK], f32, kind="ExternalInput")
    b1b = nc.dram_tensor("b1b", [128, D], f32, kind="ExternalInput")
    b2c = nc.dram_tensor("b2c", [128, 1], f32, kind="ExternalInput")
    bhb = nc.dram_tensor("bhb", [128, N_ACT], f32, kind="ExternalInput")
    ident = nc.dram_tensor("ident", [128, 128], f32, kind="ExternalInput")
    out = nc.dram_tensor("out", [N_GRAPHS, N_ACT], f32, kind="ExternalOutput")

    with tile.TileContext(nc) as tc:
        with (
            tc.tile_pool(name="const", bufs=1) as constp,
            tc.tile_pool(name="meta", bufs=1) as metap,
            tc.tile_pool(name="h1", bufs=1) as h1p,
            tc.tile_pool(name="glo", bufs=3) as glop,
            tc.tile_pool(name="ghi", bufs=2) as ghip,
            tc.tile_pool(name="ilo", bufs=3) as ilop,
            tc.tile_pool(name="ihi", bufs=2) as ihip,
            tc.tile_pool(name="sel", bufs=4) as selp,
            tc.tile_pool(name="tmp", bufs=2) as tmpp,
            tc.tile_pool(name="ps_tmpT", bufs=2, space="PSUM") as ps_tmpT,
            tc.tile_pool(name="ps_h1", bufs=2, space="PSUM") as ps_h1,
            tc.tile_pool(name="ps_misc", bufs=1, space="PSUM") as ps_misc,
            tc.tile_pool(name="dram", bufs=1, space="DRAM") as dramp,
        ):
            # constants
            W1_s = constp.tile([D, D], f32)
            nc.sync.dma_start(out=W1_s[:], in_=W1[:])
            W2_s = constp.tile([D, D], f32)
            nc.sync.dma_start(out=W2_s[:], in_=W2[:])
            Wh_s = constp.tile([D, N_ACT], f32)
            nc.sync.dma_start(out=Wh_s[:], in_=Wh[:])
            iota_s = constp.tile([128, BLK], f32)
            nc.sync.dma_start(out=iota_s[:], in_=iota[:])
            b1b_s = constp.tile([128, D], f32)
            nc.sync.dma_start(out=b1b_s[:], in_=b1b[:])
            b2c_s = constp.tile([128, 1], f32)
            nc.sync.dma_start(out=b2c_s[:], in_=b2c[:])
            bhb_s = constp.tile([128, N_ACT], f32)
            nc.sync.dma_start(out=bhb_s[:], in_=bhb[:])
            ident_s = constp.tile([128, 128], f32)
            nc.sync.dma_start(out=ident_s[:], in_=ident[:])
            w_lo_s = metap.tile([128, n_blk * A], f32)
            nc.sync.dma_start(out=w_lo_s[:], in_=w_lo[:])
            d_lo_s = metap.tile([128, n_blk * A], f32)
            nc.sync.dma_start(out=d_lo_s[:], in_=d_lo[:])
            w_hi_s = metap.tile([128, n_blk * B], f32)
            nc.sync.dma_start(out=w_hi_s[:], in_=w_hi[:])
            d_hi_s = metap.tile([128, n_blk * B], f32)
            nc.sync.dma_start(out=d_hi_s[:], in_=d_hi[:])
            sel2_s = metap.tile([128, 2 * n_blk, 128], f32)
            nc.sync.dma_start(out=sel2_s[:], in_=sel2[:])

            # gather windows, emitted lazily
            CPW = WIN // 128  # chunks per window
            lo_tiles = {}
            hi_tiles = {}

            def lo_chunk(slot):
                w, j = slot // CPW, slot % CPW
                if w not in lo_tiles:
                    it = ilop.tile([128, WIN // 16], mybir.dt.int16, tag="ilo")
                    nc.sync.dma_start(out=it[:], in_=idx_lo[w])
                    gt = glop.tile([128, CPW, D], f32, tag="glo")
                    nc.gpsimd.dma_gather(
                        out_ap=gt[:], in_ap=x[:], idxs_ap=it[:],
                        num_idxs=WIN, num_idxs_reg=WIN, elem_size=D,
                        single_packet=False,
                    )
                    lo_tiles[w] = gt
                return lo_tiles[w][:, j, :]

            def hi_chunk(slot):
                w, j = slot // CPW, slot % CPW
                if w not in hi_tiles:
                    it = ihip.tile([128, WIN // 16], mybir.dt.int16, tag="ihi")
                    nc.sync.dma_start(out=it[:], in_=idx_hi[w])
                    gt = ghip.tile([128, CPW, D], f32, tag="ghi")
                    nc.gpsimd.dma_gather(
                        out_ap=gt[:], in_ap=x[LO_LIM:, :], idxs_ap=it[:],
                        num_idxs=WIN, num_idxs_reg=WIN, elem_size=D,
                        single_packet=False,
                    )
                    hi_tiles[w] = gt
                return hi_tiles[w][:, j, :]

            h1_tiles = []
            for b in range(n_blk):
                tmpT = ps_tmpT.tile([128, BLK], f32, tag="tmpT")
                nchunks = A + B
                for k in range(nchunks):
                    if k < A:
                        slot = b * A + k
                        xg = lo_chunk(slot)
                        wv, dv = w_lo_s, d_lo_s
                    else:
                        slot = b * B + (k - A)
                        xg = hi_chunk(slot)
                        wv, dv = w_hi_s, d_hi_s
                    sel = selp.tile([128, BLK], f32, tag="sel")
                    nc.vector.tensor_scalar(
                        out=sel[:],
                        in0=iota_s[:],
                        scalar1=dv[:, slot : slot + 1],
                        scalar2=wv[:, slot : slot + 1],
                        op0=mybir.AluOpType.is_equal,
                        op1=mybir.AluOpType.mult,
                    )
                    nc.tensor.matmul(
                        out=tmpT[:], lhsT=xg, rhs=sel[:],
                        start=(k == 0), stop=(k == nchunks - 1),
                    )
                tmpT_sb = tmpp.tile([128, BLK], f32, tag="tmpT_sb")
                nc.vector.tensor_copy(out=tmpT_sb[:], in_=tmpT[:])
                for half in range(2):
                    h1pre = ps_h1.tile([128, D], f32, tag="h1pre")
                    nc.tensor.matmul(
                        out=h1pre[:],
                        lhsT=tmpT_sb[:, half * 128 : (half + 1) * 128],
                        rhs=W1_s[:],
                        start=True, stop=True,
                    )
                    h1t = h1p.tile([128, D], f32, tag=f"h1_{2*b+half}")
                    nc.vector.tensor_add(out=h1t[:], in0=h1pre[:], in1=b1b_s[:])
                    nc.scalar.activation(
                        out=h1t[:], in_=h1t[:],
                        func=mybir.ActivationFunctionType.Relu,
                    )
                    h1_tiles.append(h1t)

            # ---- layer 2: pooled accumulation ----
            Tt = ps_misc.tile([128, 128], f32, tag="Tt")
            for j in range(2 * n_blk):
                nc.tensor.matmul(
                    out=Tt[:], lhsT=h1_tiles[j][:], rhs=sel2_s[:, j, :],
                    start=(j == 0), stop=(j == 2 * n_blk - 1),
                )
            Tt_sb = tmpp.tile([128, 128], f32, tag="Tt_sb")
            nc.vector.tensor_copy(out=Tt_sb[:], in_=Tt[:])
            pooled = ps_misc.tile([128, 128], f32, tag="pooled")
            nc.tensor.matmul(out=pooled[:], lhsT=Tt_sb[:], rhs=W2_s[:], start=True, stop=True)
            pooled_sb = tmpp.tile([128, 128], f32, tag="pooled_sb")
            nc.vector.tensor_copy(out=pooled_sb[:], in_=pooled[:])

            cc_in = dramp.tile([128, 128], f32)
            cc_out = dramp.tile([128, 128], f32)
            nc.sync.dma_start(out=cc_in[:], in_=pooled_sb[:])
            nc.gpsimd.collective_compute(
                "AllReduce",
                mybir.AluOpType.add,
                replica_groups=[list(range(N_CORES))],
                ins=[cc_in.opt()],
                outs=[cc_out.opt()],
            )
            pooled_full = tmpp.tile([128, 128], f32, tag="pooled_full")
            nc.sync.dma_start(out=pooled_full[:], in_=cc_out[:])

            pooledT = ps_misc.tile([128, 128], f32, tag="pooledT")
            nc.tensor.transpose(out=pooledT[:], in_=pooled_full[:], identity=ident_s[:])
            pooledT_sb = tmpp.tile([128, 128], f32, tag="pooledT_sb")
            # copy + per-partition bias b2 (partition dim = feature)
            nc.scalar.activation(
                out=pooledT_sb[:], in_=pooledT[:],
                func=mybir.ActivationFunctionType.Identity, bias=b2c_s[:],
            )
            outp = ps_misc.tile([128, N_ACT], f32, tag="outp")
            nc.tensor.matmul(out=outp[:], lhsT=pooledT_sb[:], rhs=Wh_s[:], start=True, stop=True)
            out_sb = tmpp.tile([128, N_ACT], f32, tag="out_sb")
            nc.vector.tensor_add(out=out_sb[:], in0=outp[:], in1=bhb_s[:])
            nc.sync.dma_start(out=out[:], in_=out_sb[:])

    nc.finalize()
    if split:
        _split_multi_waits(nc)
    return nc


def _split_multi_waits(nc):
    """Bundled walrus rejects >1 sync wait per instruction; split extras onto
    same-engine NoOp carriers placed immediately before."""
    import concourse.mybir as mybir

    for f in nc.m.functions:
        for bb in f.blocks:
            insts = bb.instructions
            if not any(
                i.sync_info is not None and len(i.sync_info.on_wait) > 1
                for i in insts
            ):
                continue
            new_list = []
            for ins in insts:
                si = ins.sync_info
                if si is not None and len(si.on_wait) > 1:
                    waits = list(si.on_wait)
                    for w in waits[:-1]:
                        nop = mybir.InstNoOp(
                            name=f"waitsplit_{nc.next_id()}",
                            sync_info=mybir.SyncInfo(on_wait=[w], on_update=[]),
                            bass_nofuse=True,
                            engine=ins.engine,
                            text_hint="waitsplit",
                        )
                        new_list.append(nop)
                    si.on_wait = [waits[-1]]
                new_list.append(ins)
            bb.instructions = new_list


# ------------------------------------------------------------ device runner


def _runner_main(workdir):
    sys.path.insert(0, "/opt/trn_rl_repo")
    sys.path.insert(0, "/opt/trn_rl_repo/concourse")
    import types

    trace = os.environ.get("GCN_TRACE", "0") == "1"
    if trace:
        hookmod = types.ModuleType("antenv.axon_hooks")
        hookmod._hook = None
        hookmod.set_axon_ntff_profile_hook = lambda h: setattr(hookmod, "_hook", h)
        hookmod.get_axon_ntff_profile_hook = lambda: hookmod._hook
        sys.modules["antenv.axon_hooks"] = hookmod
        import antenv

        antenv.axon_hooks = hookmod
        try:
            from trn_agent_boot.trn_boot import _ntff_profile_via_ctypes

            hookmod.set_axon_ntff_profile_hook(
                _ntff_profile_via_ctypes("/opt/axon/libaxon_pjrt.so")
            )
        except Exception:
            trace = False

    import concourse.bass_utils as bass_utils
    from concourse.bass_utils import run_bass_kernel_spmd

    bass_utils.upload_artifacts = lambda tmpdir: tmpdir

    meta = json.load(open(os.path.join(workdir, "meta.json")))
    data = np.load(os.path.join(workdir, "inputs.npz"))
    in_maps = []
    for c in range(N_CORES):
        im = {}
        for key in data.files:
            pre = f"c{c}_"
            if key.startswith(pre):
                im[key[len(pre):]] = data[key]
        in_maps.append(im)

    nc = _build(meta)
    res = run_bass_kernel_spmd(
        nc, in_maps, core_ids=list(range(N_CORES)), trace=trace
    )
    outd = {"out": res.results[0]["out"]}
    np.savez(os.path.join(workdir, "outputs.npz"), **outd)
    info = {
        "exec_time_ns": res.exec_time_ns,
        "mean_exec_time_ns": res.mean_exec_time_ns,
        "trace": res.instructions_and_trace[1] if res.instructions_and_trace else None,
    }
    json.dump(info, open(os.path.join(workdir, "info.json"), "w"))


# ----------------------------------------------------------------- kernel()


def kernel(**inputs):
    in_maps, meta = _prep(**inputs)

    workdir = tempfile.mkdtemp(prefix="gcnkern_")
    json.dump(meta, open(os.path.join(workdir, "meta.json"), "w"))
    flat = {}
    for c, im in enumerate(in_maps):
        for k, v in im.items():
            flat[f"c{c}_{k}"] = v
    np.savez(os.path.join(workdir, "inputs.npz"), **flat)

    last_err = None
    for _attempt in range(3):
        env = dict(os.environ)
        env.pop("JAX_PLATFORMS", None)  # runner needs the TRN backend
        r = subprocess.run(
            [sys.executable, os.path.abspath(__file__), "--runner", workdir],
            capture_output=True,
            text=True,
            timeout=1800,
            env=env,
        )
        if r.returncode == 0 and os.path.exists(os.path.join(workdir, "outputs.npz")):
            break
        last_err = (r.returncode, r.stdout[-3000:], r.stderr[-3000:])
    else:
        raise RuntimeError(f"device runner failed 3x: {last_err}")

    out = np.load(os.path.join(workdir, "outputs.npz"))["out"]
    info_path = os.path.join(workdir, "info.json")
    if os.path.exists(info_path):
        kernel.last_info = json.load(open(info_path))
    return np.asarray(out, np.float32)


kernel.last_info = None


if __name__ == "__main__":
    if len(sys.argv) >= 3 and sys.argv[1] == "--runner":
        _runner_main(sys.argv[2])


# revision 6
# speedup vs baseline: 1.0005x; 1.0005x over previous
"""GCN policy network forward on 8 Trainium2 NeuronCores (Bass/Tile).

Strategy (graph/data parallel, per sharding hint):
- 128 graphs -> 16 graphs per core; node slice per core at graph boundaries.
- Layer 1: edges partitioned by dst. Per-edge rows of x are fetched with
  dma_gather (fp32, 512B rows); per 256-node dst block, PE accumulates
  tmpT[in_feat, node] += Xg_chunk^T @ sel_chunk where sel is built on DVE
  from per-edge (dst_local, norm) scalars.  Then h1 = relu(tmpT^T @ W1 + b1)
  stays resident in SBUF.
- Layer 2: edges partitioned by src (same node slice), so messages read the
  core's own h1.  Pooling is linear, so each core scatters directly into a
  per-graph pooled accumulator: Tt[feat, graph] += h1_chunk^T @ selL2_chunk
  with selL2 (multi-hot, norm/cnt weighted) precomputed on host.  One 64KB
  AllReduce combines the partial pooled sums; the head matmul runs on every
  core identically.

The device run executes in a subprocess (crash isolation + retry).
"""

import json
import os
import subprocess
import sys
import tempfile

import numpy as np

N_NODES = 50000
N_EDGES = 800000
N_GRAPHS = 128
D = 128
N_ACT = 64
N_CORES = 8
GPC = N_GRAPHS // N_CORES  # graphs per core

BLK = 256  # dst nodes per L1 block (psum free dim)
WIN = 2048  # rows per dma_gather call
LO_LIM = 32768  # int16 index limit


# ---------------------------------------------------------------- host prep


def _prep(x, edge_index, batch, W1, b1, W2, b2, W_head, b_head):
    src = np.concatenate([edge_index[0], np.arange(N_NODES, dtype=np.int64)])
    dst = np.concatenate([edge_index[1], np.arange(N_NODES, dtype=np.int64)])
    deg = np.bincount(dst, minlength=N_NODES).astype(np.float32)
    dinv = 1.0 / np.sqrt(deg)  # every node has a self-loop -> deg >= 1
    norm = (dinv[src] * dinv[dst]).astype(np.float32)

    batch = np.asarray(batch, np.int64)
    gs = np.searchsorted(batch, np.arange(N_GRAPHS + 1))
    cnt = np.bincount(batch, minlength=N_GRAPHS).astype(np.float32)
    cntc = np.maximum(cnt, 1.0)

    core_n0 = [int(gs[GPC * c]) for c in range(N_CORES)]
    core_n1 = [int(gs[GPC * (c + 1)]) for c in range(N_CORES)]
    n_blk = max(-(-(core_n1[c] - core_n0[c]) // BLK) for c in range(N_CORES))

    # ---- L1: per-core edge lists grouped by (dst block, lo/hi, src) ----
    per_core = []
    a_need = b_need = 0
    for c in range(N_CORES):
        n0, n1 = core_n0[c], core_n1[c]
        em = (dst >= n0) & (dst < n1)
        s_, d_, w_ = src[em], dst[em] - n0, norm[em]
        blk = d_ // BLK
        ishi = (s_ >= LO_LIM).astype(np.int64)
        order = np.lexsort((s_, ishi, blk))
        s_, d_, w_, blk, ishi = (a[order] for a in (s_, d_, w_, blk, ishi))
        per_core.append((s_, d_, w_, blk, ishi, n0, n1))
        for b in range(n_blk):
            nlo = int(np.count_nonzero((blk == b) & (ishi == 0)))
            nhi = int(np.count_nonzero((blk == b) & (ishi == 1)))
            a_need = max(a_need, -(-nlo // 128))
            b_need = max(b_need, -(-nhi // 128))
    A, B = a_need, b_need

    def pad_to(n, m):
        return -(-n // m) * m

    n_lo_slots = n_blk * A
    n_hi_slots = n_blk * B
    lo_rows = pad_to(n_lo_slots * 128, WIN)
    hi_rows = pad_to(n_hi_slots * 128, WIN)
    n_win_lo = lo_rows // WIN
    n_win_hi = hi_rows // WIN

    def pack_idx(idx):
        # [n] -> [128, n//16] int16, wrapped in 16 partitions, replicated x8
        a = np.asarray(idx, np.int16).reshape(-1, 16).T
        return np.tile(a, (8, 1))

    in_maps = []
    for c in range(N_CORES):
        s_, d_, w_, blk, ishi, n0, n1 = per_core[c]
        loidx = np.zeros(lo_rows, np.int64)
        low = np.zeros(lo_rows, np.float32)
        lod = np.zeros(lo_rows, np.float32)
        hiidx = np.zeros(hi_rows, np.int64)
        hiw = np.zeros(hi_rows, np.float32)
        hid = np.zeros(hi_rows, np.float32)
        for b in range(n_blk):
            mlo = (blk == b) & (ishi == 0)
            mhi = (blk == b) & (ishi == 1)
            nlo, nhi = int(np.count_nonzero(mlo)), int(np.count_nonzero(mhi))
            o = b * A * 128
            loidx[o : o + nlo] = s_[mlo]
            low[o : o + nlo] = w_[mlo]
            lod[o : o + nlo] = d_[mlo] - b * BLK
            o = b * B * 128
            hiidx[o : o + nhi] = s_[mhi] - LO_LIM
            hiw[o : o + nhi] = w_[mhi]
            hid[o : o + nhi] = d_[mhi] - b * BLK

        # stream position q -> (chunk q//128, partition q%128)
        def to_pc(a, n_slots):
            return np.ascontiguousarray(
                a[: n_slots * 128].reshape(n_slots, 128).T
            )

        im = {
            "x": None,  # filled below (shared)
            "idx_lo": np.stack(
                [pack_idx(loidx[w * WIN : (w + 1) * WIN]) for w in range(n_win_lo)]
            ),
            "idx_hi": np.stack(
                [pack_idx(hiidx[w * WIN : (w + 1) * WIN]) for w in range(n_win_hi)]
            ),
            "w_lo": to_pc(low, n_lo_slots),
            "d_lo": to_pc(lod, n_lo_slots),
            "w_hi": to_pc(hiw, n_hi_slots),
            "d_hi": to_pc(hid, n_hi_slots),
        }

        # ---- L2: multi-hot sel over the core's own nodes ----
        em2 = (src >= n0) & (src < n1)
        ln = (src[em2] - n0).astype(np.int64)
        gg = batch[dst[em2]]
        ww = (norm[em2] / cntc[gg]).astype(np.float32)
        sel2 = np.zeros((128, 2 * n_blk, 128), np.float32)
        np.add.at(sel2, (ln % 128, ln // 128, gg), ww)
        im["sel2"] = sel2
        in_maps.append(im)

    x = np.ascontiguousarray(np.asarray(x, np.float32))
    iota = np.tile(np.arange(BLK, dtype=np.float32)[None, :], (128, 1))
    b1b = np.tile(np.asarray(b1, np.float32)[None, :], (128, 1))
    b2c = np.asarray(b2, np.float32)[:, None].copy()
    bhb = np.tile(np.asarray(b_head, np.float32)[None, :], (128, 1))
    ident = np.eye(128, dtype=np.float32)
    consts = {
        "x": x,
        "W1": np.ascontiguousarray(np.asarray(W1, np.float32)),
        "W2": np.ascontiguousarray(np.asarray(W2, np.float32)),
        "Wh": np.ascontiguousarray(np.asarray(W_head, np.float32)),
        "iota": iota,
        "b1b": b1b,
        "b2c": b2c,
        "bhb": bhb,
        "ident": ident,
    }
    for im in in_maps:
        im.update(consts)

    meta = {
        "n_blk": n_blk,
        "A": A,
        "B": B,
        "n_win_lo": n_win_lo,
        "n_win_hi": n_win_hi,
    }
    return in_maps, meta


# ------------------------------------------------------------- bass builder


def _build(meta, split=True):
    import concourse.bass as bass
    import concourse.mybir as mybir
    import concourse.tile as tile
    from concourse import bacc

    n_blk, A, B = meta["n_blk"], meta["A"], meta["B"]
    n_win_lo, n_win_hi = meta["n_win_lo"], meta["n_win_hi"]
    f32 = mybir.dt.float32

    nc = bacc.Bacc(None, num_devices=N_CORES, num_swdge_queues=1)

    x = nc.dram_tensor("x", [N_NODES, D], f32, kind="ExternalInput")
    idx_lo = nc.dram_tensor("idx_lo", [n_win_lo, 128, WIN // 16], mybir.dt.int16, kind="ExternalInput")
    idx_hi = nc.dram_tensor("idx_hi", [n_win_hi, 128, WIN // 16], mybir.dt.int16, kind="ExternalInput")
    w_lo = nc.dram_tensor("w_lo", [128, n_blk * A], f32, kind="ExternalInput")
    d_lo = nc.dram_tensor("d_lo", [128, n_blk * A], f32, kind="ExternalInput")
    w_hi = nc.dram_tensor("w_hi", [128, n_blk * B], f32, kind="ExternalInput")
    d_hi = nc.dram_tensor("d_hi", [128, n_blk * B], f32, kind="ExternalInput")
    sel2 = nc.dram_tensor("sel2", [128, 2 * n_blk, 128], f32, kind="ExternalInput")
    W1 = nc.dram_tensor("W1", [D, D], f32, kind="ExternalInput")
    W2 = nc.dram_tensor("W2", [D, D], f32, kind="ExternalInput")
    Wh = nc.dram_tensor("Wh", [D, N_ACT], f32, kind="ExternalInput")
    iota = nc.dram_tensor("iota", [128, BLK], f32, kind="ExternalInput")
    b1b = nc.dram_tensor("b1b", [128, D], f32, kind="ExternalInput")
    b2c = nc.dram_tensor("b2c", [128, 1], f32, kind="ExternalInput")
    bhb = nc.dram_tensor("bhb", [128, N_ACT], f32, kind="ExternalInput")
    ident = nc.dram_tensor("ident", [128, 128], f32, kind="ExternalInput")
    out = nc.dram_tensor("out", [N_GRAPHS, N_ACT], f32, kind="ExternalOutput")

    with tile.TileContext(nc) as tc:
        with (
            tc.tile_pool(name="const", bufs=1) as constp,
            tc.tile_pool(name="meta", bufs=1) as metap,
            tc.tile_pool(name="h1", bufs=1) as h1p,
            tc.tile_pool(name="glo", bufs=3) as glop,
            tc.tile_pool(name="ghi", bufs=2) as ghip,
            tc.tile_pool(name="ilo", bufs=3) as ilop,
            tc.tile_pool(name="ihi", bufs=2) as ihip,
            tc.tile_pool(name="sel", bufs=4) as selp,
            tc.tile_pool(name="tmp", bufs=2) as tmpp,
            tc.tile_pool(name="ps_tmpT", bufs=2, space="PSUM") as ps_tmpT,
            tc.tile_pool(name="ps_h1", bufs=2, space="PSUM") as ps_h1,
            tc.tile_pool(name="ps_misc", bufs=1, space="PSUM") as ps_misc,
            tc.tile_pool(name="dram", bufs=1, space="DRAM") as dramp,
        ):
            # constants
            W1_s = constp.tile([D, D], f32)
            nc.sync.dma_start(out=W1_s[:], in_=W1[:])
            W2_s = constp.tile([D, D], f32)
            nc.sync.dma_start(out=W2_s[:], in_=W2[:])
            Wh_s = constp.tile([D, N_ACT], f32)
            nc.sync.dma_start(out=Wh_s[:], in_=Wh[:])
            iota_s = constp.tile([128, BLK], f32)
            nc.sync.dma_start(out=iota_s[:], in_=iota[:])
            b1b_s = constp.tile([128, D], f32)
            nc.sync.dma_start(out=b1b_s[:], in_=b1b[:])
            b2c_s = constp.tile([128, 1], f32)
            nc.sync.dma_start(out=b2c_s[:], in_=b2c[:])
            bhb_s = constp.tile([128, N_ACT], f32)
            nc.sync.dma_start(out=bhb_s[:], in_=bhb[:])
            ident_s = constp.tile([128, 128], f32)
            nc.sync.dma_start(out=ident_s[:], in_=ident[:])
            w_lo_s = metap.tile([128, n_blk * A], f32)
            nc.sync.dma_start(out=w_lo_s[:], in_=w_lo[:])
            d_lo_s = metap.tile([128, n_blk * A], f32)
            nc.sync.dma_start(out=d_lo_s[:], in_=d_lo[:])
            w_hi_s = metap.tile([128, n_blk * B], f32)
            nc.sync.dma_start(out=w_hi_s[:], in_=w_hi[:])
            d_hi_s = metap.tile([128, n_blk * B], f32)
            nc.sync.dma_start(out=d_hi_s[:], in_=d_hi[:])
            sel2_s = metap.tile([128, 2 * n_blk, 128], f32)
            nc.sync.dma_start(out=sel2_s[:], in_=sel2[:])

            # gather windows, emitted lazily
            CPW = WIN // 128  # chunks per window
            lo_tiles = {}
            hi_tiles = {}

            def lo_chunk(slot):
                w, j = slot // CPW, slot % CPW
                if w not in lo_tiles:
                    it = ilop.tile([128, WIN // 16], mybir.dt.int16, tag="ilo")
                    nc.sync.dma_start(out=it[:], in_=idx_lo[w])
                    gt = glop.tile([128, CPW, D], f32, tag="glo")
                    nc.gpsimd.dma_gather(
                        out_ap=gt[:], in_ap=x[:], idxs_ap=it[:],
                        num_idxs=WIN, num_idxs_reg=WIN, elem_size=D,
                        single_packet=False,
                    )
                    lo_tiles[w] = gt
                return lo_tiles[w][:, j, :]

            def hi_chunk(slot):
                w, j = slot // CPW, slot % CPW
                if w not in hi_tiles:
                    it = ihip.tile([128, WIN // 16], mybir.dt.int16, tag="ihi")
                    nc.sync.dma_start(out=it[:], in_=idx_hi[w])
                    gt = ghip.tile([128, CPW, D], f32, tag="ghi")
                    nc.gpsimd.dma_gather(
                        out_ap=gt[:], in_ap=x[LO_LIM:, :], idxs_ap=it[:],
                        num_idxs=WIN, num_idxs_reg=WIN, elem_size=D,
                        single_packet=False,
                    )
                    hi_tiles[w] = gt
                return hi_tiles[w][:, j, :]

            h1_tiles = []
            for b in range(n_blk):
                tmpT = ps_tmpT.tile([128, BLK], f32, tag="tmpT")
                nchunks = A + B
                for k in range(nchunks):
                    if k < A:
                        slot = b * A + k
                        xg = lo_chunk(slot)
                        wv, dv = w_lo_s, d_lo_s
                    else:
                        slot = b * B + (k - A)
                        xg = hi_chunk(slot)
                        wv, dv = w_hi_s, d_hi_s
                    sel = selp.tile([128, BLK], f32, tag="sel")
                    nc.vector.tensor_scalar(
                        out=sel[:],
                        in0=iota_s[:],
                        scalar1=dv[:, slot : slot + 1],
                        scalar2=wv[:, slot : slot + 1],
                        op0=mybir.AluOpType.is_equal,
                        op1=mybir.AluOpType.mult,
                    )
                    nc.tensor.matmul(
                        out=tmpT[:], lhsT=xg, rhs=sel[:],
                        start=(k == 0), stop=(k == nchunks - 1),
                    )
                tmpT_sb = tmpp.tile([128, BLK], f32, tag="tmpT_sb")
                nc.vector.tensor_copy(out=tmpT_sb[:], in_=tmpT[:])
                for half in range(2):
                    h1pre = ps_h1.tile([128, D], f32, tag="h1pre")
                    nc.tensor.matmul(
                        out=h1pre[:],
                        lhsT=tmpT_sb[:, half * 128 : (half + 1) * 128],
                        rhs=W1_s[:],
                        start=True, stop=True,
                    )
                    h1t = h1p.tile([128, D], f32, tag=f"h1_{2*b+half}")
                    nc.vector.tensor_add(out=h1t[:], in0=h1pre[:], in1=b1b_s[:])
                    nc.scalar.activation(
                        out=h1t[:], in_=h1t[:],
                        func=mybir.ActivationFunctionType.Relu,
                    )
                    h1_tiles.append(h1t)

            # ---- layer 2: pooled accumulation ----
            Tt = ps_misc.tile([128, 128], f32, tag="Tt")
            for j in range(2 * n_blk):
                nc.tensor.matmul(
                    out=Tt[:], lhsT=h1_tiles[j][:], rhs=sel2_s[:, j, :],
                    start=(j == 0), stop=(j == 2 * n_blk - 1),
                )
            Tt_sb = tmpp.tile([128, 128], f32, tag="Tt_sb")
            nc.vector.tensor_copy(out=Tt_sb[:], in_=Tt[:])
            pooled = ps_misc.tile([128, 128], f32, tag="pooled")
            nc.tensor.matmul(out=pooled[:], lhsT=Tt_sb[:], rhs=W2_s[:], start=True, stop=True)
            pooled_sb = tmpp.tile([128, 128], f32, tag="pooled_sb")
            nc.vector.tensor_copy(out=pooled_sb[:], in_=pooled[:])

            cc_in = dramp.tile([128, 128], f32)
            cc_out = dramp.tile([128, 128], f32)
            nc.sync.dma_start(out=cc_in[:], in_=pooled_sb[:])
            nc.gpsimd.collective_compute(
                "AllReduce",
                mybir.AluOpType.add,
                replica_groups=[list(range(N_CORES))],
                ins=[cc_in.opt()],
                outs=[cc_out.opt()],
            )
            pooled_full = tmpp.tile([128, 128], f32, tag="pooled_full")
            nc.sync.dma_start(out=pooled_full[:], in_=cc_out[:])

            pooledT = ps_misc.tile([128, 128], f32, tag="pooledT")
            nc.tensor.transpose(out=pooledT[:], in_=pooled_full[:], identity=ident_s[:])
            pooledT_sb = tmpp.tile([128, 128], f32, tag="pooledT_sb")
            # copy + per-partition bias b2 (partition dim = feature)
            nc.scalar.activation(
                out=pooledT_sb[:], in_=pooledT[:],
                func=mybir.ActivationFunctionType.Identity, bias=b2c_s[:],
            )
            outp = ps_misc.tile([128, N_ACT], f32, tag="outp")
            nc.tensor.matmul(out=outp[:], lhsT=pooledT_sb[:], rhs=Wh_s[:], start=True, stop=True)
            out_sb = tmpp.tile([128, N_ACT], f32, tag="out_sb")
            nc.vector.tensor_add(out=out_sb[:], in0=outp[:], in1=bhb_s[:])
            nc.sync.dma_start(out=out[:], in_=out_sb[:])

    nc.finalize()
    if split:
        _split_multi_waits(nc)
    return nc


def _split_multi_waits(nc):
    """Bundled walrus rejects >1 sync wait per instruction; split extras onto
    same-engine NoOp carriers placed immediately before."""
    import concourse.mybir as mybir

    for f in nc.m.functions:
        for bb in f.blocks:
            insts = bb.instructions
            if not any(
                i.sync_info is not None and len(i.sync_info.on_wait) > 1
                for i in insts
            ):
                continue
            new_list = []
            for ins in insts:
                si = ins.sync_info
                if si is not None and len(si.on_wait) > 1:
                    waits = list(si.on_wait)
                    for w in waits[:-1]:
                        nop = mybir.InstNoOp(
                            name=f"waitsplit_{nc.next_id()}",
                            sync_info=mybir.SyncInfo(on_wait=[w], on_update=[]),
                            bass_nofuse=True,
                            engine=ins.engine,
                            text_hint="waitsplit",
                        )
                        new_list.append(nop)
                    si.on_wait = [waits[-1]]
                new_list.append(ins)
            bb.instructions = new_list


# ------------------------------------------------------------ device runner


def _runner_main(workdir):
    sys.path.insert(0, "/opt/trn_rl_repo")
    sys.path.insert(0, "/opt/trn_rl_repo/concourse")
    import types

    trace = os.environ.get("GCN_TRACE", "0") == "1"
    if trace:
        hookmod = types.ModuleType("antenv.axon_hooks")
        hookmod._hook = None
        hookmod.set_axon_ntff_profile_hook = lambda h: setattr(hookmod, "_hook", h)
        hookmod.get_axon_ntff_profile_hook = lambda: hookmod._hook
        sys.modules["antenv.axon_hooks"] = hookmod
        import antenv

        antenv.axon_hooks = hookmod
        try:
            from trn_agent_boot.trn_boot import _ntff_profile_via_ctypes

            hookmod.set_axon_ntff_profile_hook(
                _ntff_profile_via_ctypes("/opt/axon/libaxon_pjrt.so")
            )
        except Exception:
            trace = False

    import concourse.bass_utils as bass_utils
    from concourse.bass_utils import run_bass_kernel_spmd

    bass_utils.upload_artifacts = lambda tmpdir: tmpdir

    meta = json.load(open(os.path.join(workdir, "meta.json")))
    data = np.load(os.path.join(workdir, "inputs.npz"))
    in_maps = []
    for c in range(N_CORES):
        im = {}
        for key in data.files:
            pre = f"c{c}_"
            if key.startswith(pre):
                im[key[len(pre):]] = data[key]
        in_maps.append(im)

    nc = _build(meta)
    res = run_bass_kernel_spmd(
        nc, in_maps, core_ids=list(range(N_CORES)), trace=trace
    )
    outd = {"out": res.results[0]["out"]}
    np.savez(os.path.join(workdir, "outputs.npz"), **outd)
    info = {
        "exec_time_ns": res.exec_time_ns,
        "mean_exec_time_ns": res.mean_exec_time_ns,
        "trace": res.instructions_and_trace[1] if res.instructions_and_trace else None,
    }
    if trace and res.instructions_and_trace:
        insts = res.instructions_and_trace[0]
        agg = {}
        for i in insts:
            key = f"{i.engine}:{(i.name or i.label or '?').split('.')[0]}"
            n, d = agg.get(key, (0, 0))
            agg[key] = (n + 1, d + i.duration)
        eng = {}
        for i in insts:
            n, d = eng.get(str(i.engine), (0, 0))
            eng[str(i.engine)] = (n + 1, d + i.duration)
        info["engine_busy_ns"] = {k: v[1] for k, v in eng.items()}
        info["top_ops"] = sorted(
            ((k, v[0], v[1]) for k, v in agg.items()), key=lambda t: -t[2]
        )[:12]
    json.dump(info, open(os.path.join(workdir, "info.json"), "w"))


# ----------------------------------------------------------------- kernel()


def kernel(**inputs):
    in_maps, meta = _prep(**inputs)

    workdir = tempfile.mkdtemp(prefix="gcnkern_")
    json.dump(meta, open(os.path.join(workdir, "meta.json"), "w"))
    flat = {}
    for c, im in enumerate(in_maps):
        for k, v in im.items():
            flat[f"c{c}_{k}"] = v
    np.savez(os.path.join(workdir, "inputs.npz"), **flat)

    last_err = None
    for _attempt in range(3):
        env = dict(os.environ)
        env.pop("JAX_PLATFORMS", None)  # runner needs the TRN backend
        r = subprocess.run(
            [sys.executable, os.path.abspath(__file__), "--runner", workdir],
            capture_output=True,
            text=True,
            timeout=1800,
            env=env,
        )
        if r.returncode == 0 and os.path.exists(os.path.join(workdir, "outputs.npz")):
            break
        last_err = (r.returncode, r.stdout[-3000:], r.stderr[-3000:])
    else:
        raise RuntimeError(f"device runner failed 3x: {last_err}")

    out = np.load(os.path.join(workdir, "outputs.npz"))["out"]
    info_path = os.path.join(workdir, "info.json")
    if os.path.exists(info_path):
        kernel.last_info = json.load(open(info_path))
    return np.asarray(out, np.float32)


kernel.last_info = None


if __name__ == "__main__":
    if len(sys.argv) >= 3 and sys.argv[1] == "--runner":
        _runner_main(sys.argv[2])


# revision 7
# speedup vs baseline: 1.4426x; 1.4418x over previous
"""GCN policy network forward on 8 Trainium2 NeuronCores (Bass/Tile).

Strategy (graph/data parallel, per sharding hint):
- 128 graphs -> 16 graphs per core; node slice per core at graph boundaries.
- Layer 1: edges partitioned by dst. Per-edge rows of x are fetched with
  dma_gather (fp32, 512B rows); per 256-node dst block, PE accumulates
  tmpT[in_feat, node] += Xg_chunk^T @ sel_chunk where sel is built on DVE
  from per-edge (dst_local, norm) scalars.  Then h1 = relu(tmpT^T @ W1 + b1)
  stays resident in SBUF.
- Layer 2: edges partitioned by src (same node slice), so messages read the
  core's own h1.  Pooling is linear, so each core scatters directly into a
  per-graph pooled accumulator: Tt[feat, graph] += h1_chunk^T @ selL2_chunk
  with selL2 (multi-hot, norm/cnt weighted) precomputed on host.  One 64KB
  AllReduce combines the partial pooled sums; the head matmul runs on every
  core identically.

The device run executes in a subprocess (crash isolation + retry).
"""

import json
import os
import subprocess
import sys
import tempfile

import numpy as np

N_NODES = 50000
N_EDGES = 800000
N_GRAPHS = 128
D = 128
N_ACT = 64
N_CORES = 8
GPC = N_GRAPHS // N_CORES  # graphs per core

BLK = 256  # dst nodes per L1 block (psum free dim)
WIN = 2048  # rows per dma_gather call
LO_LIM = 32768  # int16 index limit


# ---------------------------------------------------------------- host prep


def _prep(x, edge_index, batch, W1, b1, W2, b2, W_head, b_head):
    src = np.concatenate([edge_index[0], np.arange(N_NODES, dtype=np.int64)])
    dst = np.concatenate([edge_index[1], np.arange(N_NODES, dtype=np.int64)])
    deg = np.bincount(dst, minlength=N_NODES).astype(np.float32)
    dinv = 1.0 / np.sqrt(deg)  # every node has a self-loop -> deg >= 1
    norm = (dinv[src] * dinv[dst]).astype(np.float32)

    batch = np.asarray(batch, np.int64)
    gs = np.searchsorted(batch, np.arange(N_GRAPHS + 1))
    cnt = np.bincount(batch, minlength=N_GRAPHS).astype(np.float32)
    cntc = np.maximum(cnt, 1.0)

    core_n0 = [int(gs[GPC * c]) for c in range(N_CORES)]
    core_n1 = [int(gs[GPC * (c + 1)]) for c in range(N_CORES)]
    n_blk = max(-(-(core_n1[c] - core_n0[c]) // BLK) for c in range(N_CORES))

    # ---- L1: per-core edge lists grouped by (dst block, lo/hi, src) ----
    per_core = []
    a_need = b_need = 0
    for c in range(N_CORES):
        n0, n1 = core_n0[c], core_n1[c]
        em = (dst >= n0) & (dst < n1)
        s_, d_, w_ = src[em], dst[em] - n0, norm[em]
        blk = d_ // BLK
        ishi = (s_ >= LO_LIM).astype(np.int64)
        order = np.lexsort((s_, ishi, blk))
        s_, d_, w_, blk, ishi = (a[order] for a in (s_, d_, w_, blk, ishi))
        per_core.append((s_, d_, w_, blk, ishi, n0, n1))
        for b in range(n_blk):
            nlo = int(np.count_nonzero((blk == b) & (ishi == 0)))
            nhi = int(np.count_nonzero((blk == b) & (ishi == 1)))
            a_need = max(a_need, -(-nlo // 128))
            b_need = max(b_need, -(-nhi // 128))
    A, B = a_need, b_need

    def pad_to(n, m):
        return -(-n // m) * m

    n_lo_slots = n_blk * A
    n_hi_slots = n_blk * B
    lo_rows = pad_to(n_lo_slots * 128, WIN)
    hi_rows = pad_to(n_hi_slots * 128, WIN)
    n_win_lo = lo_rows // WIN
    n_win_hi = hi_rows // WIN

    def pack_idx(idx):
        # [n] -> [128, n//16] int16, wrapped in 16 partitions, replicated x8
        a = np.asarray(idx, np.int16).reshape(-1, 16).T
        return np.tile(a, (8, 1))

    in_maps = []
    for c in range(N_CORES):
        s_, d_, w_, blk, ishi, n0, n1 = per_core[c]
        loidx = np.zeros(lo_rows, np.int64)
        low = np.zeros(lo_rows, np.float32)
        lod = np.zeros(lo_rows, np.float32)
        hiidx = np.zeros(hi_rows, np.int64)
        hiw = np.zeros(hi_rows, np.float32)
        hid = np.zeros(hi_rows, np.float32)
        for b in range(n_blk):
            mlo = (blk == b) & (ishi == 0)
            mhi = (blk == b) & (ishi == 1)
            nlo, nhi = int(np.count_nonzero(mlo)), int(np.count_nonzero(mhi))
            o = b * A * 128
            loidx[o : o + nlo] = s_[mlo]
            low[o : o + nlo] = w_[mlo]
            lod[o : o + nlo] = d_[mlo] - b * BLK
            o = b * B * 128
            hiidx[o : o + nhi] = s_[mhi] - LO_LIM
            hiw[o : o + nhi] = w_[mhi]
            hid[o : o + nhi] = d_[mhi] - b * BLK

        # stream position q -> (chunk q//128, partition q%128)
        def to_pc(a, n_slots):
            return np.ascontiguousarray(
                a[: n_slots * 128].reshape(n_slots, 128).T
            )

        # combined per-block slot layout: slot b*(A+B)+c, c<A -> lo, else hi
        wl, dl = to_pc(low, n_lo_slots), to_pc(lod, n_lo_slots)
        wh, dh = to_pc(hiw, n_hi_slots), to_pc(hid, n_hi_slots)
        w_all = np.zeros((128, n_blk * (A + B)), np.float32)
        d_all = np.zeros((128, n_blk * (A + B)), np.float32)
        for b in range(n_blk):
            o = b * (A + B)
            w_all[:, o : o + A] = wl[:, b * A : (b + 1) * A]
            d_all[:, o : o + A] = dl[:, b * A : (b + 1) * A]
            w_all[:, o + A : o + A + B] = wh[:, b * B : (b + 1) * B]
            d_all[:, o + A : o + A + B] = dh[:, b * B : (b + 1) * B]
        im = {
            "x": None,  # filled below (shared)
            "idx_lo": np.stack(
                [pack_idx(loidx[w * WIN : (w + 1) * WIN]) for w in range(n_win_lo)]
            ),
            "idx_hi": np.stack(
                [pack_idx(hiidx[w * WIN : (w + 1) * WIN]) for w in range(n_win_hi)]
            ),
            "w_all": w_all,
            "d_all": d_all,
        }

        # ---- L2: multi-hot sel over the core's own nodes ----
        em2 = (src >= n0) & (src < n1)
        ln = (src[em2] - n0).astype(np.int64)
        gg = batch[dst[em2]]
        ww = (norm[em2] / cntc[gg]).astype(np.float32)
        sel2 = np.zeros((128, 2 * n_blk, 128), np.float32)
        np.add.at(sel2, (ln % 128, ln // 128, gg), ww)
        im["sel2"] = sel2
        in_maps.append(im)

    x = np.ascontiguousarray(np.asarray(x, np.float32))
    iota = np.tile(np.arange(BLK, dtype=np.float32)[None, :], (128, 1))
    b1b = np.tile(np.asarray(b1, np.float32)[None, :], (128, 1))
    b2c = np.asarray(b2, np.float32)[:, None].copy()
    bhb = np.tile(np.asarray(b_head, np.float32)[None, :], (128, 1))
    ident = np.eye(128, dtype=np.float32)
    consts = {
        "x": x,
        "W1": np.ascontiguousarray(np.asarray(W1, np.float32)),
        "W2": np.ascontiguousarray(np.asarray(W2, np.float32)),
        "Wh": np.ascontiguousarray(np.asarray(W_head, np.float32)),
        "iota": iota,
        "b1b": b1b,
        "b2c": b2c,
        "bhb": bhb,
        "ident": ident,
    }
    for im in in_maps:
        im.update(consts)

    meta = {
        "n_blk": n_blk,
        "A": A,
        "B": B,
        "n_win_lo": n_win_lo,
        "n_win_hi": n_win_hi,
    }
    return in_maps, meta


# ------------------------------------------------------------- bass builder


def _build(meta, split=True):
    import concourse.bass as bass
    import concourse.mybir as mybir
    import concourse.tile as tile
    from concourse import bacc

    n_blk, A, B = meta["n_blk"], meta["A"], meta["B"]
    n_win_lo, n_win_hi = meta["n_win_lo"], meta["n_win_hi"]
    f32 = mybir.dt.float32

    nc = bacc.Bacc(None, num_devices=N_CORES, num_swdge_queues=2)

    x = nc.dram_tensor("x", [N_NODES, D], f32, kind="ExternalInput")
    idx_lo = nc.dram_tensor("idx_lo", [n_win_lo, 128, WIN // 16], mybir.dt.int16, kind="ExternalInput")
    idx_hi = nc.dram_tensor("idx_hi", [n_win_hi, 128, WIN // 16], mybir.dt.int16, kind="ExternalInput")
    bf16 = mybir.dt.bfloat16
    w_all = nc.dram_tensor("w_all", [128, n_blk * (A + B)], f32, kind="ExternalInput")
    d_all = nc.dram_tensor("d_all", [128, n_blk * (A + B)], bf16, kind="ExternalInput")
    sel2 = nc.dram_tensor("sel2", [128, 2 * n_blk, 128], bf16, kind="ExternalInput")
    W1 = nc.dram_tensor("W1", [D, D], f32, kind="ExternalInput")
    W2 = nc.dram_tensor("W2", [D, D], f32, kind="ExternalInput")
    Wh = nc.dram_tensor("Wh", [D, N_ACT], f32, kind="ExternalInput")
    iota = nc.dram_tensor("iota", [128, BLK], bf16, kind="ExternalInput")
    b1b = nc.dram_tensor("b1b", [128, D], f32, kind="ExternalInput")
    b2c = nc.dram_tensor("b2c", [128, 1], f32, kind="ExternalInput")
    bhb = nc.dram_tensor("bhb", [128, N_ACT], f32, kind="ExternalInput")
    ident = nc.dram_tensor("ident", [128, 128], f32, kind="ExternalInput")
    out = nc.dram_tensor("out", [N_GRAPHS, N_ACT], f32, kind="ExternalOutput")

    with tile.TileContext(nc) as tc:
        with (
            tc.tile_pool(name="const", bufs=1) as constp,
            tc.tile_pool(name="meta", bufs=1) as metap,
            tc.tile_pool(name="h1", bufs=1) as h1p,
            tc.tile_pool(name="glo", bufs=3) as glop,
            tc.tile_pool(name="ghi", bufs=2) as ghip,
            tc.tile_pool(name="ilo", bufs=3) as ilop,
            tc.tile_pool(name="ihi", bufs=2) as ihip,
            tc.tile_pool(name="sel", bufs=4) as selp,
            tc.tile_pool(name="tmp", bufs=2) as tmpp,
            tc.tile_pool(name="ps_tmpT", bufs=2, space="PSUM") as ps_tmpT,
            tc.tile_pool(name="ps_h1", bufs=2, space="PSUM") as ps_h1,
            tc.tile_pool(name="ps_misc", bufs=1, space="PSUM") as ps_misc,
            tc.tile_pool(name="dram", bufs=1, space="DRAM") as dramp,
        ):
            # constants
            W1_s = constp.tile([D, D], f32)
            nc.sync.dma_start(out=W1_s[:], in_=W1[:])
            W2_s = constp.tile([D, D], f32)
            nc.sync.dma_start(out=W2_s[:], in_=W2[:])
            Wh_s = constp.tile([D, N_ACT], f32)
            nc.sync.dma_start(out=Wh_s[:], in_=Wh[:])
            iota_s = constp.tile([128, BLK], bf16)
            nc.sync.dma_start(out=iota_s[:], in_=iota[:])
            b1b_s = constp.tile([128, D], f32)
            nc.sync.dma_start(out=b1b_s[:], in_=b1b[:])
            b2c_s = constp.tile([128, 1], f32)
            nc.sync.dma_start(out=b2c_s[:], in_=b2c[:])
            bhb_s = constp.tile([128, N_ACT], f32)
            nc.sync.dma_start(out=bhb_s[:], in_=bhb[:])
            ident_s = constp.tile([128, 128], f32)
            nc.sync.dma_start(out=ident_s[:], in_=ident[:])
            w_all_s = metap.tile([128, n_blk * (A + B)], f32)
            nc.sync.dma_start(out=w_all_s[:], in_=w_all[:])
            d_all_s = metap.tile([128, n_blk * (A + B)], bf16)
            nc.sync.dma_start(out=d_all_s[:], in_=d_all[:])
            sel2_s = metap.tile([128, 2 * n_blk, 128], bf16)
            nc.sync.dma_start(out=sel2_s[:], in_=sel2[:])

            # gather windows, emitted lazily
            CPW = WIN // 128  # chunks per window
            lo_tiles = {}
            hi_tiles = {}

            def lo_chunk(slot):
                w, j = slot // CPW, slot % CPW
                if w not in lo_tiles:
                    it = ilop.tile([128, WIN // 16], mybir.dt.int16, tag="ilo")
                    nc.sync.dma_start(out=it[:], in_=idx_lo[w])
                    gt = glop.tile([128, CPW, D], f32, tag="glo")
                    nc.gpsimd.dma_gather(
                        out_ap=gt[:], in_ap=x[:], idxs_ap=it[:],
                        num_idxs=WIN, num_idxs_reg=WIN, elem_size=D,
                        single_packet=False,
                    )
                    lo_tiles[w] = gt
                return lo_tiles[w][:, j, :]

            def hi_chunk(slot):
                w, j = slot // CPW, slot % CPW
                if w not in hi_tiles:
                    it = ihip.tile([128, WIN // 16], mybir.dt.int16, tag="ihi")
                    nc.sync.dma_start(out=it[:], in_=idx_hi[w])
                    gt = ghip.tile([128, CPW, D], f32, tag="ghi")
                    nc.gpsimd.dma_gather(
                        out_ap=gt[:], in_ap=x[LO_LIM:, :], idxs_ap=it[:],
                        num_idxs=WIN, num_idxs_reg=WIN, elem_size=D,
                        single_packet=False,
                    )
                    hi_tiles[w] = gt
                return hi_tiles[w][:, j, :]

            h1_tiles = []
            nchunks = A + B
            for b in range(n_blk):
                tmpT = ps_tmpT.tile([128, BLK], f32, tag="tmpT")
                # one-hot block selector for all chunks of this block at once
                sel01 = selp.tile([128, nchunks, BLK], bf16, tag="sel01")
                o = b * nchunks
                nc.vector.tensor_tensor(
                    out=sel01[:],
                    in0=iota_s[:].unsqueeze(1).to_broadcast([128, nchunks, BLK]),
                    in1=d_all_s[:, o : o + nchunks].unsqueeze(2).to_broadcast(
                        [128, nchunks, BLK]
                    ),
                    op=mybir.AluOpType.is_equal,
                )
                for k in range(nchunks):
                    if k < A:
                        xg = lo_chunk(b * A + k)
                    else:
                        xg = hi_chunk(b * B + (k - A))
                    sidx = o + k
                    xgs = selp.tile([128, D], bf16, tag="xgs")
                    nc.scalar.activation(
                        out=xgs[:], in_=xg,
                        func=mybir.ActivationFunctionType.Copy,
                        scale=w_all_s[:, sidx : sidx + 1],
                    )
                    nc.tensor.matmul(
                        out=tmpT[:], lhsT=xgs[:], rhs=sel01[:, k, :],
                        start=(k == 0), stop=(k == nchunks - 1),
                    )
                tmpT_sb = tmpp.tile([128, BLK], f32, tag="tmpT_sb")
                nc.vector.tensor_copy(out=tmpT_sb[:], in_=tmpT[:])
                for half in range(2):
                    h1pre = ps_h1.tile([128, D], f32, tag="h1pre")
                    nc.tensor.matmul(
                        out=h1pre[:],
                        lhsT=tmpT_sb[:, half * 128 : (half + 1) * 128],
                        rhs=W1_s[:],
                        start=True, stop=True,
                    )
                    h1a = tmpp.tile([128, D], f32, tag="h1a")
                    nc.vector.tensor_add(out=h1a[:], in0=h1pre[:], in1=b1b_s[:])
                    h1t = h1p.tile([128, D], bf16, tag=f"h1_{2*b+half}")
                    nc.scalar.activation(
                        out=h1t[:], in_=h1a[:],
                        func=mybir.ActivationFunctionType.Relu,
                    )
                    h1_tiles.append(h1t)

            # ---- layer 2: pooled accumulation ----
            Tt = ps_misc.tile([128, 128], f32, tag="Tt")
            for j in range(2 * n_blk):
                nc.tensor.matmul(
                    out=Tt[:], lhsT=h1_tiles[j][:], rhs=sel2_s[:, j, :],
                    start=(j == 0), stop=(j == 2 * n_blk - 1),
                )
            Tt_sb = tmpp.tile([128, 128], f32, tag="Tt_sb")
            nc.vector.tensor_copy(out=Tt_sb[:], in_=Tt[:])
            pooled = ps_misc.tile([128, 128], f32, tag="pooled")
            nc.tensor.matmul(out=pooled[:], lhsT=Tt_sb[:], rhs=W2_s[:], start=True, stop=True)
            pooled_sb = tmpp.tile([128, 128], f32, tag="pooled_sb")
            nc.vector.tensor_copy(out=pooled_sb[:], in_=pooled[:])

            cc_in = dramp.tile([128, 128], f32)
            cc_out = dramp.tile([128, 128], f32)
            nc.sync.dma_start(out=cc_in[:], in_=pooled_sb[:])
            nc.gpsimd.collective_compute(
                "AllReduce",
                mybir.AluOpType.add,
                replica_groups=[list(range(N_CORES))],
                ins=[cc_in.opt()],
                outs=[cc_out.opt()],
            )
            pooled_full = tmpp.tile([128, 128], f32, tag="pooled_full")
            nc.sync.dma_start(out=pooled_full[:], in_=cc_out[:])

            pooledT = ps_misc.tile([128, 128], f32, tag="pooledT")
            nc.tensor.transpose(out=pooledT[:], in_=pooled_full[:], identity=ident_s[:])
            pooledT_sb = tmpp.tile([128, 128], f32, tag="pooledT_sb")
            # copy + per-partition bias b2 (partition dim = feature)
            nc.scalar.activation(
                out=pooledT_sb[:], in_=pooledT[:],
                func=mybir.ActivationFunctionType.Identity, bias=b2c_s[:],
            )
            outp = ps_misc.tile([128, N_ACT], f32, tag="outp")
            nc.tensor.matmul(out=outp[:], lhsT=pooledT_sb[:], rhs=Wh_s[:], start=True, stop=True)
            out_sb = tmpp.tile([128, N_ACT], f32, tag="out_sb")
            nc.vector.tensor_add(out=out_sb[:], in0=outp[:], in1=bhb_s[:])
            nc.sync.dma_start(out=out[:], in_=out_sb[:])

    nc.finalize()
    _assign_gather_queues(nc)
    if split:
        _split_multi_waits(nc)
    return nc


def _assign_gather_queues(nc):
    """Spread dma_gather desc-gen over both SWDGE queues, consistently with
    the DMASW sem lane Tile assigned (sem lane k is driven from queue k%2)."""
    for f in nc.m.functions:
        for bb in f.blocks:
            for ins in bb.instructions:
                if type(ins).__name__ != "InstDMAGatherAnt":
                    continue
                si = ins.sync_info
                if si is None:
                    continue
                for u in si.on_update or []:
                    nm = u.ant_name or ""
                    if nm.startswith("DMASW"):
                        ins.queue_num = int(nm[5:].split("_")[0]) % 2
                        break


def _split_multi_waits(nc):
    """Bundled walrus rejects >1 sync wait per instruction; split extras onto
    same-engine NoOp carriers placed immediately before."""
    import concourse.mybir as mybir

    for f in nc.m.functions:
        for bb in f.blocks:
            insts = bb.instructions
            if not any(
                i.sync_info is not None and len(i.sync_info.on_wait) > 1
                for i in insts
            ):
                continue
            new_list = []
            for ins in insts:
                si = ins.sync_info
                if si is not None and len(si.on_wait) > 1:
                    waits = list(si.on_wait)
                    for w in waits[:-1]:
                        nop = mybir.InstNoOp(
                            name=f"waitsplit_{nc.next_id()}",
                            sync_info=mybir.SyncInfo(on_wait=[w], on_update=[]),
                            bass_nofuse=True,
                            engine=ins.engine,
                            text_hint="waitsplit",
                        )
                        new_list.append(nop)
                    si.on_wait = [waits[-1]]
                new_list.append(ins)
            bb.instructions = new_list


# ------------------------------------------------------------ device runner


def _runner_main(workdir):
    sys.path.insert(0, "/opt/trn_rl_repo")
    sys.path.insert(0, "/opt/trn_rl_repo/concourse")
    import types

    trace = os.environ.get("GCN_TRACE", "0") == "1"
    if trace:
        hookmod = types.ModuleType("antenv.axon_hooks")
        hookmod._hook = None
        hookmod.set_axon_ntff_profile_hook = lambda h: setattr(hookmod, "_hook", h)
        hookmod.get_axon_ntff_profile_hook = lambda: hookmod._hook
        sys.modules["antenv.axon_hooks"] = hookmod
        import antenv

        antenv.axon_hooks = hookmod
        try:
            from trn_agent_boot.trn_boot import _ntff_profile_via_ctypes

            hookmod.set_axon_ntff_profile_hook(
                _ntff_profile_via_ctypes("/opt/axon/libaxon_pjrt.so")
            )
        except Exception:
            trace = False

    import concourse.bass_utils as bass_utils
    from concourse.bass_utils import run_bass_kernel_spmd

    bass_utils.upload_artifacts = lambda tmpdir: tmpdir

    meta = json.load(open(os.path.join(workdir, "meta.json")))
    data = np.load(os.path.join(workdir, "inputs.npz"))
    in_maps = []
    for c in range(N_CORES):
        im = {}
        for key in data.files:
            pre = f"c{c}_"
            if key.startswith(pre):
                im[key[len(pre):]] = data[key]
        in_maps.append(im)

    import ml_dtypes

    for im in in_maps:
        for k in ("d_all", "sel2", "iota"):
            im[k] = np.ascontiguousarray(im[k]).astype(ml_dtypes.bfloat16)

    nc = _build(meta)
    res = run_bass_kernel_spmd(
        nc, in_maps, core_ids=list(range(N_CORES)), trace=trace
    )
    outd = {"out": res.results[0]["out"]}
    np.savez(os.path.join(workdir, "outputs.npz"), **outd)
    info = {
        "exec_time_ns": res.exec_time_ns,
        "mean_exec_time_ns": res.mean_exec_time_ns,
        "trace": res.instructions_and_trace[1] if res.instructions_and_trace else None,
    }
    if trace and res.instructions_and_trace:
        insts = res.instructions_and_trace[0]
        agg = {}
        for i in insts:
            key = f"{i.engine}:{(i.name or i.label or '?').split('.')[0]}"
            n, d = agg.get(key, (0, 0))
            agg[key] = (n + 1, d + i.duration)
        eng = {}
        for i in insts:
            n, d = eng.get(str(i.engine), (0, 0))
            eng[str(i.engine)] = (n + 1, d + i.duration)
        info["engine_busy_ns"] = {k: v[1] for k, v in eng.items()}
        info["top_ops"] = sorted(
            ((k, v[0], v[1]) for k, v in agg.items()), key=lambda t: -t[2]
        )[:12]
    json.dump(info, open(os.path.join(workdir, "info.json"), "w"))


# ----------------------------------------------------------------- kernel()


def kernel(**inputs):
    in_maps, meta = _prep(**inputs)

    workdir = tempfile.mkdtemp(prefix="gcnkern_")
    json.dump(meta, open(os.path.join(workdir, "meta.json"), "w"))
    flat = {}
    for c, im in enumerate(in_maps):
        for k, v in im.items():
            flat[f"c{c}_{k}"] = v
    np.savez(os.path.join(workdir, "inputs.npz"), **flat)

    last_err = None
    for _attempt in range(3):
        env = dict(os.environ)
        env.pop("JAX_PLATFORMS", None)  # runner needs the TRN backend
        r = subprocess.run(
            [sys.executable, os.path.abspath(__file__), "--runner", workdir],
            capture_output=True,
            text=True,
            timeout=1800,
            env=env,
        )
        if r.returncode == 0 and os.path.exists(os.path.join(workdir, "outputs.npz")):
            break
        last_err = (r.returncode, r.stdout[-3000:], r.stderr[-3000:])
    else:
        raise RuntimeError(f"device runner failed 3x: {last_err}")

    out = np.load(os.path.join(workdir, "outputs.npz"))["out"]
    info_path = os.path.join(workdir, "info.json")
    if os.path.exists(info_path):
        kernel.last_info = json.load(open(info_path))
    return np.asarray(out, np.float32)


kernel.last_info = None


if __name__ == "__main__":
    if len(sys.argv) >= 3 and sys.argv[1] == "--runner":
        _runner_main(sys.argv[2])
